# revision 2
# baseline (speedup 1.0000x reference)
"""DigitCaps dynamic-routing kernel for Trainium2 (8 NeuronCores, Bass/Tile).

Strategy (pure batch data-parallelism, 64 batch rows per core):
  u_hat (B,1152,10,16) is NEVER materialized. Per routing iteration:

    s[b,(o,p)]   = sum_k x2[k,b] * (e ⊙ Ws)[k,(o,p)]      (72 fp16 K-tile matmuls)
    M2[(o,p),k]  = sum_b v[b,(o,p)] * x3[b,k]             (fp8 matmuls; 160 (o,p)
                   rows live as 128 + 32-padded-to-128 in two PSUM banks)
    agree[o,i]   = Sel^T @ (W ⊙ M2)  with (p,s)-reduction on the PE
                   (one tensor_tensor product + one fp8 DoubleRow Sel matmul
                   per 384-col chunk, accumulated over s in PSUM)

  PIPELINED ROUTING (this version): the agree phase is split into 3 i-blocks
  of 384.  Each block's agree partial is AllReduced independently (3 small
  [10,384] collectives per iteration instead of one [10,1152]), and the next
  iteration's b-update / exp / c-broadcast / s-matmuls for that i-block start
  as soon as its AllReduce lands — overlapping the remaining blocks' agree
  compute and collectives.  This removes the ~25-35us of all-engine idle per
  iteration that the monolithic AllReduce caused.

  UNNORMALIZED SOFTMAX: to decouple blocks, c_ij = exp(b)/Z is NOT normalized
  before the s-matmul; the rhs uses exp(b) directly (b in [-0.03, 0.21], so
  exp(b)~1 is safe in fp16) and the global 1/Z_o is applied at the end:
  a diag(1/Z) [10,10] matmul against a ones[10,64] stationary broadcasts
  1/Z_o to a [64,10] tile, which folds into the squash scalars exactly
  (v = s_un*squn*zb2 / ((1+squn*zb2)*sqrt(squn*zb2+eps)) * zb1).  This is
  mathematically identical to the reference softmax (the shift is 0).

Precision: s-matmuls in fp16 for all 3 iterations; the agree path (v, x3,
products) in fp8e4m3, W-product operand in fp8e3m4 — this error only perturbs
the routing logits b_ij and stays well inside the 2e-2 budget.

Scalar-engine activation tables: exp and sqrt live in different table sets, so
each switch costs ~1.3us. Dummy activations prefetch the upcoming set while
the scalar engine is idle, keeping table loads off the serial chains.
"""
import sys

sys.path.insert(0, "/opt/trn_rl_repo")

import numpy as np
import ml_dtypes

# ---- problem constants (hardcoded per harness contract) ----
B, I, S, O, P = 512, 1152, 8, 10, 16
IS = I * S            # 9216  contraction size, k = s*I + i
OP = O * P            # 160
NCORES = 8
BL = B // NCORES      # 64 batch rows per core
KT = IS // 128        # 72 K-tiles
IC = I // 128         # 9 i-chunks per s-group
FB = 384              # free-chunk width of the agree pipeline (I = 3*FB)
NBLK = 3              # i-blocks per routing iteration (pipelined AllReduce)
QB = IC // NBLK       # 3 k-chunks of 128 per (s, block)
F32MAX = 512          # PSUM bank width in f32 elements

SEL_LAG = 3           # chunks between product issue and its Sel matmul

_CACHE = {}


def _build_module():
    import concourse.bass as bass
    import concourse.mybir as mybir
    import concourse.tile as tile
    from concourse import bacc

    f32 = mybir.dt.float32
    bf16 = mybir.dt.bfloat16
    fp16 = mybir.dt.float16
    fp8 = mybir.dt.float8e4
    fp8w = mybir.dt.float8e3
    MUL = mybir.AluOpType.mult
    ADD = mybir.AluOpType.add
    DR = mybir.MatmulPerfMode.DoubleRow
    AF = mybir.ActivationFunctionType

    nc = bacc.Bacc(
        "TRN2",
        target_bir_lowering=False,
        debug=False,
        num_devices=NCORES,
    )

    # ---- I/O ----
    x2h_d = nc.dram_tensor("x2h", [128, KT, BL], fp16, kind="ExternalInput")
    wsh_d = nc.dram_tensor("wsh", [128, KT, OP], fp16, kind="ExternalInput")
    x3f_d = nc.dram_tensor("x3f", [BL, IS], fp8, kind="ExternalInput")
    wt2_d = nc.dram_tensor("wt2", [128, 2, IS], fp8w, kind="ExternalInput")
    sel8_d = nc.dram_tensor("sel8", [128, 2, 32], fp8, kind="ExternalInput")
    b2h_d = nc.dram_tensor("b2h", [O, OP], fp16, kind="ExternalInput")
    ones64_d = nc.dram_tensor("ones64", [O, BL], fp16, kind="ExternalInput")
    id10_d = nc.dram_tensor("id10", [O, O], fp16, kind="ExternalInput")
    vout_d = nc.dram_tensor("vout", [BL, OP], f32, kind="ExternalOutput")

    with tile.TileContext(nc) as tc:
        with (
            tc.tile_pool(name="const", bufs=1) as const,
            tc.tile_pool(name="rhsbig", bufs=3) as rhsp,
            tc.tile_pool(name="prod", bufs=8) as prodp,
            tc.tile_pool(name="m2c", bufs=2) as m2cp,
            tc.tile_pool(name="cexp", bufs=2) as cexpp,
            tc.tile_pool(name="ring1", bufs=1) as ring1,
            tc.tile_pool(name="ring2", bufs=2) as ring2,
            tc.tile_pool(name="psA", bufs=1, space="PSUM") as psA,
            tc.tile_pool(name="psM", bufs=2, space="PSUM") as psM,
            tc.tile_pool(name="psG", bufs=2, space="PSUM") as psG,
            tc.tile_pool(name="psC", bufs=1, space="PSUM") as psC,
            tc.tile_pool(name="dram", bufs=1, space="DRAM") as dram,
        ):
            # ---------- persistent tiles ----------
            x2h = const.tile([128, KT, BL], fp16)
            wsh = const.tile([128, KT, OP], fp16)
            x3f = const.tile([BL, IS], fp8)
            wt2 = const.tile([128, 2, IS], fp8w)
            sel8 = const.tile([128, 2, 32], fp8)
            b2h = const.tile([O, OP], fp16)
            ones64 = const.tile([O, BL], fp16)
            id10 = const.tile([O, O], fp16)
            v8 = const.tile([BL, 2 * 128], fp8)  # squash out + zero pad

            # Sized warmup AllReduce input staged FIRST on the sync queue so
            # the warmup collective runs right after the entry barrier,
            # leaving the cc stream free for the real agree AllReduces.
            warm_sb = const.tile([O, FB], f32)
            nc.vector.memset(warm_sb[:], 0.0)
            warm_in = dram.tile([O, FB], f32, tag="warm_in")
            warm_out = dram.tile([O, FB], f32, tag="warm_out")
            nc.sync.dma_start(warm_in[:], warm_sb[:])
            nc.gpsimd.collective_compute(
                "AllReduce",
                ADD,
                replica_groups=[list(range(NCORES))],
                ins=[warm_in.opt()],
                outs=[warm_out.opt()],
            )

            # ---------- load inputs (3 parallel queues) ----------
            # scalar/gpsimd: wsh alternating s-groups (the iter0 critical
            # stream); sync: x2h then agree-phase operands.
            nc.gpsimd.dma_start(sel8[:], sel8_d[:])
            nc.gpsimd.dma_start(b2h[:], b2h_d[:])
            nc.gpsimd.dma_start(ones64[:], ones64_d[:])
            nc.gpsimd.dma_start(id10[:], id10_d[:])
            for q0 in range(0, 3):
                ks = slice(q0 * 3, (q0 + 1) * 3)
                nc.scalar.dma_start(wsh[:, ks, :], wsh_d[:, ks, :])
            for s in range(1, S):
                ks = slice(s * IC, (s + 1) * IC)
                eng = nc.scalar if s % 2 == 0 else nc.gpsimd
                eng.dma_start(wsh[:, ks, :], wsh_d[:, ks, :])
            for s in range(S):
                ks = slice(s * IC, (s + 1) * IC)
                nc.sync.dma_start(x2h[:, ks, :], x2h_d[:, ks, :])
            nc.sync.dma_start(x3f[:], x3f_d[:])
            JCH = 3 * FB
            for c0 in range(0, IS, JCH):
                cs = slice(c0, c0 + JCH)
                nc.sync.dma_start(wt2[:, :, cs], wt2_d[:, :, cs])

            # zero pad region of v8 (persists across iterations)
            nc.vector.memset(v8[:, OP:], 0.0)

            # bias APs for activation (float biases need pre-registered consts)
            zero_b = const.tile([128, 1], f32)
            eps_b = const.tile([128, 1], f32)
            nc.vector.memset(zero_b[:], 0.0)
            nc.vector.memset(eps_b[:], 1e-8)
            # scratch for activation-table prefetch dummies
            tscr = const.tile([1, 2], f32)
            nc.vector.memset(tscr[:], 1.0)

            # ---------- iter0 s-matmul phase (uniform c = 1/I) ----------
            s_ps = psA.tile([BL, O, P], f32, tag="sps")
            for s in range(S):
                if s == 0:
                    # prefetch sqrt table while the PE streams
                    tsd = ring2.tile([1, 2], f32, tag="tsd0")
                    nc.scalar.activation(tsd[:], tscr[:], AF.Sqrt, bias=eps_b[:1])
                for icx in range(IC):
                    k = s * IC + icx
                    nc.tensor.matmul(
                        s_ps[:],
                        x2h[:, k, :],
                        wsh[:, k, :],
                        start=(k == 0),
                        stop=(k == KT - 1),
                    )

            # ---------- iter0 squash (exact: fold 1/I) ----------
            s_sb = ring1.tile([BL, O, P], f32, tag="s_sb0")
            nc.vector.tensor_scalar_mul(s_sb[:], s_ps[:], 1.0 / I)
            s2 = ring1.tile([BL, O, P], f32, tag="s20")
            nc.vector.tensor_tensor(s2[:], s_sb[:], s_sb[:], MUL)
            sq = ring1.tile([BL, O], f32, tag="sq0")
            nc.vector.tensor_reduce(sq[:], s2[:], axis=mybir.AxisListType.X, op=ADD)
            sqs = ring1.tile([BL, O], f32, tag="sqs0")
            nc.scalar.activation(sqs[:], sq[:], AF.Sqrt, bias=eps_b[:BL])
            den = ring1.tile([BL, O], f32, tag="den0")
            nc.vector.scalar_tensor_tensor(
                den[:], sq[:], 1.0, sqs[:], op0=ADD, op1=MUL
            )
            rec = ring1.tile([BL, O], f32, tag="rec0")
            nc.vector.reciprocal(rec[:], den[:])
            tfac = ring1.tile([BL, O], f32, tag="tfac0")
            nc.vector.tensor_tensor(tfac[:], sq[:], rec[:], MUL)
            nc.vector.tensor_tensor(
                v8[:, 0:OP].rearrange("b (o p) -> b o p", o=O),
                s_sb[:],
                tfac[:, :, None].to_broadcast([BL, O, P]),
                MUL,
            )

            bT_prev = None  # SBUF (10, I) f32 routing logits

            for it in range(2):  # routing updates
                # =========== agree phase, block-major, pipelined AR ==========
                agARs = []
                ag_outs = []
                for ib in range(NBLK):
                    ag = psG.tile([32, F32MAX], f32, tag="ag")
                    pend = []

                    def emit_sel(pe):
                        pa, s_idx = pe
                        nc.tensor.matmul(
                            ag[:, 0:FB],
                            sel8[:],
                            pa[:],
                            start=(s_idx == 0),
                            stop=(s_idx == S - 1),
                            perf_mode=DR,
                        )

                    for s in range(S):
                        j = s * NBLK + ib
                        fs = slice(j * FB, (j + 1) * FB)
                        m2 = psM.tile([128, 2, F32MAX], f32, tag="m2")
                        nc.tensor.matmul(
                            m2[:, 0, 0:FB],
                            v8[:, 0:128],
                            x3f[:, fs],
                            start=True,
                            stop=True,
                        )
                        nc.tensor.matmul(
                            m2[:, 1, 0:FB],
                            v8[:, 128:256],
                            x3f[:, fs],
                            start=True,
                            stop=True,
                        )
                        if ib == 0 and s == 0:
                            # prefetch exp table while the PE streams the agree
                            # phase (squash's sqrt is done)
                            ted = ring2.tile([1, 2], f32, tag=f"ted{it}")
                            nc.scalar.activation(
                                ted[:], tscr[:], AF.Exp, bias=zero_b[:1]
                            )
                        if len(pend) >= SEL_LAG:
                            emit_sel(pend.pop(0))
                        pa = prodp.tile([128, 2, FB], fp8, tag="prod")
                        if s in (1, 4):
                            m2c = m2cp.tile([128, 2, FB], bf16, tag="m2c")
                            nc.scalar.copy(out=m2c[:], in_=m2[:, :, 0:FB])
                            nc.gpsimd.tensor_tensor(pa[:], m2c[:], wt2[:, :, fs], MUL)
                        else:
                            nc.vector.tensor_tensor(
                                pa[:], m2[:, :, 0:FB], wt2[:, :, fs], MUL
                            )
                        pend.append((pa, s))
                    while pend:
                        emit_sel(pend.pop(0))

                    # agree partial for this block -> DRAM -> AllReduce
                    agP = ring1.tile([O, FB], f32, tag=f"agP{it}{ib}")
                    nc.scalar.copy(out=agP[:], in_=ag[0:O, 0:FB])
                    ag_in = dram.tile([O, FB], f32, tag=f"agin{it}{ib}")
                    ag_out = dram.tile([O, FB], f32, tag=f"agout{it}{ib}")
                    nc.sync.dma_start(ag_in[:], agP[:])
                    nc.gpsimd.collective_compute(
                        "AllReduce",
                        ADD,
                        replica_groups=[list(range(NCORES))],
                        ins=[ag_in.opt()],
                        outs=[ag_out.opt()],
                    )
                    ag_outs.append(ag_out)

                # read-backs AFTER all stagings/triggers so the sync queue
                # never blocks a later block's staging behind an AR wait
                for ib in range(NBLK):
                    agAR = ring1.tile([O, FB], f32, tag=f"agAR{it}{ib}")
                    nc.sync.dma_start(agAR[:], ag_outs[ib][:])
                    agARs.append(agAR)

                # ====== per-block: b-update, exp, c-broadcast, next-s ======
                nit = it + 1
                last = nit == 2
                s_ps = psA.tile([BL, O, P], f32, tag="sps")
                cexp = cexpp.tile([128, IC, OP], fp16, tag="cexp")
                eT16 = ring1.tile([O, I], fp16, tag=f"eT{it}")
                bT = ring1.tile([O, I], f32, tag=f"bT{it}")
                zps = []
                first_mm = True
                for ib in range(NBLK):
                    blk = slice(ib * FB, (ib + 1) * FB)
                    if bT_prev is None:
                        nc.vector.tensor_scalar_mul(
                            bT[:, blk], agARs[ib][:], 1.0 / B
                        )
                    else:
                        nc.vector.scalar_tensor_tensor(
                            bT[:, blk],
                            agARs[ib][:],
                            1.0 / B,
                            bT_prev[:, blk],
                            op0=MUL,
                            op1=ADD,
                        )
                    zp = ring1.tile([O, 1], f32, tag=f"zp{it}{ib}")
                    nc.scalar.activation(
                        eT16[:, blk],
                        bT[:, blk],
                        AF.Exp,
                        bias=zero_b[:O],
                        accum_out=zp[:],
                    )
                    zps.append(zp)
                    if ib == NBLK - 1:
                        # prefetch sqrt for the upcoming squash
                        tsd = ring2.tile([1, 2], f32, tag=f"tsd{nit}")
                        nc.scalar.activation(
                            tsd[:], tscr[:], AF.Sqrt, bias=eps_b[:1]
                        )

                    # unnormalized c broadcast across p: ce[m,(o,p)] = e[o, i]
                    ce = psC.tile([128, QB * OP], f32, tag="ce")
                    for q in range(QB):
                        icx = ib * QB + q
                        nc.tensor.matmul(
                            ce[:, q * OP : (q + 1) * OP],
                            eT16[:, icx * 128 : (icx + 1) * 128],
                            b2h[:],
                            start=True,
                            stop=True,
                        )
                    nc.scalar.copy(
                        out=cexp[:, ib * QB : (ib + 1) * QB, :],
                        in_=ce[:, 0 : QB * OP].rearrange("p (q n) -> p q n", n=OP),
                    )

                    # next-iteration s-matmuls for this i-block
                    for s in range(S):
                        ks = slice(s * IC + ib * QB, s * IC + (ib + 1) * QB)
                        rhs = rhsp.tile([128, QB, OP], fp16, tag="rhs16")
                        nc.vector.tensor_tensor(
                            rhs[:], wsh[:, ks, :], cexp[:, ib * QB : (ib + 1) * QB, :], MUL
                        )
                        for q in range(QB):
                            k = s * IC + ib * QB + q
                            nc.tensor.matmul(
                                s_ps[:],
                                x2h[:, k, :],
                                rhs[:, q, :],
                                start=first_mm,
                                stop=(ib == NBLK - 1 and s == S - 1 and q == QB - 1),
                            )
                            first_mm = False
                bT_prev = bT

                # ====== Z combine + 1/Z broadcast to [BL, O] via the PE ======
                zs1 = ring1.tile([O, 1], f32, tag=f"zs1{it}")
                nc.vector.tensor_tensor(zs1[:], zps[0][:], zps[1][:], ADD)
                zsum = ring1.tile([O, 1], f32, tag=f"zsum{it}")
                nc.vector.tensor_tensor(zsum[:], zs1[:], zps[2][:], ADD)
                zrec = ring1.tile([O, 1], f32, tag=f"zrec{it}")
                nc.vector.reciprocal(zrec[:], zsum[:])
                diag = ring1.tile([O, O], fp16, tag=f"diag{it}")
                nc.vector.tensor_scalar_mul(diag[:], id10[:], zrec[:])
                zb_ps = psC.tile([BL, O], f32, tag="ce")
                nc.tensor.matmul(zb_ps[:], ones64[:], diag[:], start=True, stop=True)
                zb1 = ring1.tile([BL, O], f32, tag=f"zb1{it}")
                nc.vector.tensor_copy(zb1[:], zb_ps[:])
                zb2 = ring1.tile([BL, O], f32, tag=f"zb2{it}")
                nc.vector.tensor_tensor(zb2[:], zb1[:], zb1[:], MUL)

                # ====== squash of the unnormalized s ======
                s_sb = ring1.tile([BL, O, P], f32, tag=f"s_sb{nit}")
                nc.vector.tensor_copy(s_sb[:], s_ps[:])
                s2 = ring1.tile([BL, O, P], f32, tag=f"s2{nit}")
                nc.vector.tensor_tensor(s2[:], s_sb[:], s_sb[:], MUL)
                squn = ring1.tile([BL, O], f32, tag=f"squn{nit}")
                nc.vector.tensor_reduce(
                    squn[:], s2[:], axis=mybir.AxisListType.X, op=ADD
                )
                sq = ring1.tile([BL, O], f32, tag=f"sq{nit}")
                nc.vector.tensor_tensor(sq[:], squn[:], zb2[:], MUL)
                sqs = ring1.tile([BL, O], f32, tag=f"sqs{nit}")
                nc.scalar.activation(sqs[:], sq[:], AF.Sqrt, bias=eps_b[:BL])
                den = ring1.tile([BL, O], f32, tag=f"den{nit}")
                nc.vector.scalar_tensor_tensor(
                    den[:], sq[:], 1.0, sqs[:], op0=ADD, op1=MUL
                )
                rec = ring1.tile([BL, O], f32, tag=f"rec{nit}")
                nc.vector.reciprocal(rec[:], den[:])
                tfac = ring1.tile([BL, O], f32, tag=f"tfac{nit}")
                nc.vector.tensor_tensor(tfac[:], sq[:], rec[:], MUL)
                tf2 = ring1.tile([BL, O], f32, tag=f"tf2{nit}")
                nc.vector.tensor_tensor(tf2[:], tfac[:], zb1[:], MUL)

                if last:
                    v_sb = ring1.tile([BL, O, P], f32, tag="v_sb")
                    nc.vector.tensor_tensor(
                        v_sb[:],
                        s_sb[:],
                        tf2[:, :, None].to_broadcast([BL, O, P]),
                        MUL,
                    )
                    nc.sync.dma_start(vout_d[:], v_sb[:])
                else:
                    nc.vector.tensor_tensor(
                        v8[:, 0:OP].rearrange("b (o p) -> b o p", o=O),
                        s_sb[:],
                        tf2[:, :, None].to_broadcast([BL, O, P]),
                        MUL,
                    )

    nc.compile()
    return nc


def _get_module():
    if "nc" not in _CACHE:
        _CACHE["nc"] = _build_module()
    return _CACHE["nc"]


def _prep_inputs(x, W):
    """Host-side relayouts (free: not counted in HW exec time)."""
    f8 = ml_dtypes.float8_e4m3
    f8w = ml_dtypes.float8_e3m4
    f16 = np.float16
    x = np.ascontiguousarray(np.asarray(x, np.float32))
    W = np.ascontiguousarray(np.asarray(W, np.float32))

    x2 = x.transpose(2, 1, 0).reshape(IS, B)           # [(s,i), b]
    x2t = x2.reshape(KT, 128, B).transpose(1, 0, 2)    # [p, k, b]
    x3 = x.transpose(0, 2, 1).reshape(B, IS)           # [b, (s,i)]
    ws = W.transpose(3, 0, 1, 2).reshape(IS, OP)       # [(s,i), (o,p)]
    wst = ws.reshape(KT, 128, OP).transpose(1, 0, 2)   # [p, k, (o,p)]
    wt = W.transpose(1, 2, 3, 0).reshape(OP, IS)       # [(o,p), (s,i)]

    wt2 = np.zeros((128, 2, IS), np.float32)
    wt2[:, 0, :] = wt[0:128]
    wt2[0:32, 1, :] = wt[128:160]

    sel8 = np.zeros((128, 2, 32), np.float32)
    for o in range(O):
        sel8[o * P : (o + 1) * P, 0, o] = 1.0
    for r in range(32):
        sel8[r, 1, (128 + r) // P] = 1.0

    b2 = np.zeros((O, OP), np.float32)
    for o in range(O):
        b2[o, o * P : (o + 1) * P] = 1.0

    shared = {
        "wsh": np.ascontiguousarray(wst).astype(f16),
        "wt2": wt2.astype(f8w),
        "sel8": sel8.astype(f8),
        "b2h": b2.astype(f16),
        "ones64": np.ones((O, BL), f16),
        "id10": np.eye(O, dtype=f16),
    }
    in_maps = []
    for c in range(NCORES):
        bs = slice(c * BL, (c + 1) * BL)
        m = dict(shared)
        m["x2h"] = np.ascontiguousarray(x2t[:, :, bs]).astype(f16)
        m["x3f"] = np.ascontiguousarray(x3[bs]).astype(f8)
        in_maps.append(m)
    return in_maps


def run(x, W, trace=False, tmpdir=None):
    import jax
    from concourse import bass_utils

    try:
        jax.config.update("jax_compilation_cache_dir", "/tmp/jax_neff_cache")
        jax.config.update("jax_persistent_cache_min_compile_time_secs", 1.0)
    except Exception:
        pass

    nc = _get_module()
    in_maps = _prep_inputs(x, W)
    res = bass_utils.run_bass_kernel_spmd(
        nc, in_maps, core_ids=list(range(NCORES)), trace=trace, tmpdir=tmpdir
    )
    v = np.concatenate([res.results[c]["vout"] for c in range(NCORES)], axis=0)
    return v.reshape(B, O, P).astype(np.float32), res


def kernel(x, W):
    v, _ = run(x, W)
    return v


# revision 5
# speedup vs baseline: 1.0528x; 1.0528x over previous
"""DigitCaps dynamic-routing kernel for Trainium2 (8 NeuronCores, Bass/Tile).

Strategy (pure batch data-parallelism, 64 batch rows per core):
  u_hat (B,1152,10,16) is NEVER materialized. Per routing iteration:

    s[b,(o,p)]   = sum_k x2[k,b] * (e ⊙ Ws)[k,(o,p)]      (72 fp16 K-tile matmuls)
    M2[(o,p),k]  = sum_b v[b,(o,p)] * x3[b,k]             (fp8 matmuls; 160 (o,p)
                   rows live as 128 + 32-padded-to-128 in two PSUM banks)
    agree[o,i]   = Sel^T @ (W ⊙ M2)  with (p,s)-reduction on the PE
                   (one tensor_tensor product + one fp8 DoubleRow Sel matmul
                   per 384-col chunk, accumulated over s in PSUM)

  PIPELINED ROUTING (this version): the agree phase is split into 3 i-blocks
  of 384.  Each block's agree partial is AllReduced independently (3 small
  [10,384] collectives per iteration instead of one [10,1152]), and the next
  iteration's b-update / exp / c-broadcast / s-matmuls for that i-block start
  as soon as its AllReduce lands — overlapping the remaining blocks' agree
  compute and collectives.  This removes the ~25-35us of all-engine idle per
  iteration that the monolithic AllReduce caused.

  UNNORMALIZED SOFTMAX: to decouple blocks, c_ij = exp(b)/Z is NOT normalized
  before the s-matmul; the rhs uses exp(b) directly (b in [-0.03, 0.21], so
  exp(b)~1 is safe in fp16) and the global 1/Z_o is applied at the end:
  a diag(1/Z) [10,10] matmul against a ones[10,64] stationary broadcasts
  1/Z_o to a [64,10] tile, which folds into the squash scalars exactly
  (v = s_un*squn*zb2 / ((1+squn*zb2)*sqrt(squn*zb2+eps)) * zb1).  This is
  mathematically identical to the reference softmax (the shift is 0).

Precision: s-matmuls in fp16 for all 3 iterations; the agree path (v, x3,
products) in fp8e4m3, W-product operand in fp8e3m4 — this error only perturbs
the routing logits b_ij and stays well inside the 2e-2 budget.

Scalar-engine activation tables: exp and sqrt live in different table sets, so
each switch costs ~1.3us. Dummy activations prefetch the upcoming set while
the scalar engine is idle, keeping table loads off the serial chains.
"""
import sys

sys.path.insert(0, "/opt/trn_rl_repo")

import numpy as np
import ml_dtypes

# ---- problem constants (hardcoded per harness contract) ----
B, I, S, O, P = 512, 1152, 8, 10, 16
IS = I * S            # 9216  contraction size, k = s*I + i
OP = O * P            # 160
NCORES = 8
BL = B // NCORES      # 64 batch rows per core
KT = IS // 128        # 72 K-tiles
IC = I // 128         # 9 i-chunks per s-group
FB = 384              # free-chunk width of the agree pipeline (I = 3*FB)
NBLK = 3              # i-blocks per routing iteration (pipelined AllReduce)
QB = IC // NBLK       # 3 k-chunks of 128 per (s, block)
F32MAX = 512          # PSUM bank width in f32 elements

SEL_LAG = 3           # chunks between product issue and its Sel matmul

_CACHE = {}


def _build_module():
    import concourse.bass as bass
    import concourse.mybir as mybir
    import concourse.tile as tile
    from concourse import bacc

    f32 = mybir.dt.float32
    bf16 = mybir.dt.bfloat16
    fp16 = mybir.dt.float16
    fp8 = mybir.dt.float8e4
    fp8w = mybir.dt.float8e3
    MUL = mybir.AluOpType.mult
    ADD = mybir.AluOpType.add
    DR = mybir.MatmulPerfMode.DoubleRow
    AF = mybir.ActivationFunctionType

    nc = bacc.Bacc(
        "TRN2",
        target_bir_lowering=False,
        debug=False,
        num_devices=NCORES,
    )

    # ---- I/O ----
    x2h_d = nc.dram_tensor("x2h", [128, KT, BL], fp16, kind="ExternalInput")
    wsh_d = nc.dram_tensor("wsh", [128, KT, OP], fp16, kind="ExternalInput")
    x3f_d = nc.dram_tensor("x3f", [BL, IS], fp8, kind="ExternalInput")
    wt2_d = nc.dram_tensor("wt2", [128, 2, IS], fp8w, kind="ExternalInput")
    sel8_d = nc.dram_tensor("sel8", [128, 2, 32], fp8, kind="ExternalInput")
    b2h_d = nc.dram_tensor("b2h", [O, OP], fp16, kind="ExternalInput")
    ones64_d = nc.dram_tensor("ones64", [O, BL], fp16, kind="ExternalInput")
    id10_d = nc.dram_tensor("id10", [O, O], fp16, kind="ExternalInput")
    vout_d = nc.dram_tensor("vout", [BL, OP], f32, kind="ExternalOutput")

    with tile.TileContext(nc) as tc:
        with (
            tc.tile_pool(name="const", bufs=1) as const,
            tc.tile_pool(name="rhsbig", bufs=3) as rhsp,
            tc.tile_pool(name="prod", bufs=8) as prodp,
            tc.tile_pool(name="m2c", bufs=2) as m2cp,
            tc.tile_pool(name="cexp", bufs=2) as cexpp,
            tc.tile_pool(name="ring1", bufs=1) as ring1,
            tc.tile_pool(name="ring2", bufs=2) as ring2,
            tc.tile_pool(name="psA", bufs=1, space="PSUM") as psA,
            tc.tile_pool(name="psM", bufs=2, space="PSUM") as psM,
            tc.tile_pool(name="psG", bufs=2, space="PSUM") as psG,
            tc.tile_pool(name="psC", bufs=1, space="PSUM") as psC,
            tc.tile_pool(name="dram", bufs=1, space="DRAM") as dram,
        ):
            # ---------- persistent tiles ----------
            x2h = const.tile([128, KT, BL], fp16)
            wsh = const.tile([128, KT, OP], fp16)
            x3f = const.tile([BL, IS], fp8)
            wt2 = const.tile([128, 2, IS], fp8w)
            sel8 = const.tile([128, 2, 32], fp8)
            b2h = const.tile([O, OP], fp16)
            ones64 = const.tile([O, BL], fp16)
            id10 = const.tile([O, O], fp16)
            v8 = const.tile([BL, 2 * 128], fp8)  # squash out + zero pad

            # ---------- load inputs (3 parallel queues) ----------
            # scalar/gpsimd: wsh alternating s-groups (the iter0 critical
            # stream); sync: x2h then agree-phase operands.
            nc.gpsimd.dma_start(sel8[:], sel8_d[:])
            nc.gpsimd.dma_start(b2h[:], b2h_d[:])
            nc.gpsimd.dma_start(ones64[:], ones64_d[:])
            nc.gpsimd.dma_start(id10[:], id10_d[:])
            for q0 in range(0, 3):
                ks = slice(q0 * 3, (q0 + 1) * 3)
                nc.scalar.dma_start(wsh[:, ks, :], wsh_d[:, ks, :])
            for s in range(1, S):
                ks = slice(s * IC, (s + 1) * IC)
                eng = nc.scalar if s % 2 == 0 else nc.gpsimd
                eng.dma_start(wsh[:, ks, :], wsh_d[:, ks, :])
            for s in range(S):
                ks = slice(s * IC, (s + 1) * IC)
                nc.sync.dma_start(x2h[:, ks, :], x2h_d[:, ks, :])
            nc.sync.dma_start(x3f[:], x3f_d[:])
            JCH = 3 * FB
            for c0 in range(0, IS, JCH):
                cs = slice(c0, c0 + JCH)
                nc.sync.dma_start(wt2[:, :, cs], wt2_d[:, :, cs])

            # zero pad region of v8 (persists across iterations)
            nc.vector.memset(v8[:, OP:], 0.0)

            # bias APs for activation (float biases need pre-registered consts)
            zero_b = const.tile([128, 1], f32)
            eps_b = const.tile([128, 1], f32)
            nc.vector.memset(zero_b[:], 0.0)
            nc.vector.memset(eps_b[:], 1e-8)
            # scratch for activation-table prefetch dummies
            tscr = const.tile([1, 2], f32)
            nc.vector.memset(tscr[:], 1.0)

            # ---------- iter0 s-matmul phase (uniform c = 1/I) ----------
            s_ps = psA.tile([BL, O, P], f32, tag="sps")
            for s in range(S):
                if s == 0:
                    # prefetch sqrt table while the PE streams
                    tsd = ring2.tile([1, 2], f32, tag="tsd0")
                    nc.scalar.activation(tsd[:], tscr[:], AF.Sqrt, bias=eps_b[:1])
                for icx in range(IC):
                    k = s * IC + icx
                    nc.tensor.matmul(
                        s_ps[:],
                        x2h[:, k, :],
                        wsh[:, k, :],
                        start=(k == 0),
                        stop=(k == KT - 1),
                    )

            # ---------- iter0 squash (exact: fold 1/I) ----------
            s_sb = ring1.tile([BL, O, P], f32, tag="s_sb0")
            nc.vector.tensor_scalar_mul(s_sb[:], s_ps[:], 1.0 / I)
            s2 = ring1.tile([BL, O, P], f32, tag="s20")
            nc.vector.tensor_tensor(s2[:], s_sb[:], s_sb[:], MUL)
            sq = ring1.tile([BL, O], f32, tag="sq0")
            nc.vector.tensor_reduce(sq[:], s2[:], axis=mybir.AxisListType.X, op=ADD)
            sqs = ring1.tile([BL, O], f32, tag="sqs0")
            nc.scalar.activation(sqs[:], sq[:], AF.Sqrt, bias=eps_b[:BL])
            den = ring1.tile([BL, O], f32, tag="den0")
            nc.vector.scalar_tensor_tensor(
                den[:], sq[:], 1.0, sqs[:], op0=ADD, op1=MUL
            )
            rec = ring1.tile([BL, O], f32, tag="rec0")
            nc.vector.reciprocal(rec[:], den[:])
            tfac = ring1.tile([BL, O], f32, tag="tfac0")
            nc.vector.tensor_tensor(tfac[:], sq[:], rec[:], MUL)
            nc.vector.tensor_tensor(
                v8[:, 0:OP].rearrange("b (o p) -> b o p", o=O),
                s_sb[:],
                tfac[:, :, None].to_broadcast([BL, O, P]),
                MUL,
            )

            bT_prev = None  # SBUF (10, I) f32 routing logits

            for it in range(2):  # routing updates
                # =========== agree phase, block-major, pipelined AR ==========
                # Two AllReduces per iteration: A covers i-blocks {0,1}, B
                # covers block {2}.  Collective triggers BLOCK the gpsimd
                # queue until the previous collective completes, so block 2's
                # products must stay off gpsimd (they'd stall behind trig A).
                agPfull = ring1.tile([O, I], f32, tag=f"agP{it}")
                agARfull = ring1.tile([O, I], f32, tag=f"agAR{it}")
                ag_inA = dram.tile([O, 2 * FB], f32, tag=f"aginA{it}")
                ag_outA = dram.tile([O, 2 * FB], f32, tag=f"agoutA{it}")
                ag_inB = dram.tile([O, FB], f32, tag=f"aginB{it}")
                ag_outB = dram.tile([O, FB], f32, tag=f"agoutB{it}")
                for ib in range(NBLK):
                    ag = psG.tile([32, F32MAX], f32, tag="ag")
                    pend = []

                    def emit_sel(pe):
                        pa, s_idx = pe
                        nc.tensor.matmul(
                            ag[:, 0:FB],
                            sel8[:],
                            pa[:],
                            start=(s_idx == 0),
                            stop=(s_idx == S - 1),
                            perf_mode=DR,
                        )

                    for s in range(S):
                        j = s * NBLK + ib
                        fs = slice(j * FB, (j + 1) * FB)
                        m2 = psM.tile([128, 2, F32MAX], f32, tag="m2")
                        nc.tensor.matmul(
                            m2[:, 0, 0:FB],
                            v8[:, 0:128],
                            x3f[:, fs],
                            start=True,
                            stop=True,
                        )
                        nc.tensor.matmul(
                            m2[:, 1, 0:FB],
                            v8[:, 128:256],
                            x3f[:, fs],
                            start=True,
                            stop=True,
                        )
                        if ib == 0 and s == 0:
                            # prefetch exp table while the PE streams the agree
                            # phase (squash's sqrt is done)
                            ted = ring2.tile([1, 2], f32, tag=f"ted{it}")
                            nc.scalar.activation(
                                ted[:], tscr[:], AF.Exp, bias=zero_b[:1]
                            )
                        if len(pend) >= SEL_LAG:
                            emit_sel(pend.pop(0))
                        pa = prodp.tile([128, 2, FB], fp8, tag="prod")
                        if ib < 2 and s in (1, 4):
                            m2c = m2cp.tile([128, 2, FB], bf16, tag="m2c")
                            nc.scalar.copy(out=m2c[:], in_=m2[:, :, 0:FB])
                            nc.gpsimd.tensor_tensor(pa[:], m2c[:], wt2[:, :, fs], MUL)
                        else:
                            nc.vector.tensor_tensor(
                                pa[:], m2[:, :, 0:FB], wt2[:, :, fs], MUL
                            )
                        pend.append((pa, s))
                    while pend:
                        emit_sel(pend.pop(0))

                    blk = slice(ib * FB, (ib + 1) * FB)
                    nc.scalar.copy(out=agPfull[:, blk], in_=ag[0:O, 0:FB])
                    if ib == 1:
                        nc.sync.dma_start(ag_inA[:], agPfull[:, 0 : 2 * FB])
                        nc.gpsimd.collective_compute(
                            "AllReduce",
                            ADD,
                            replica_groups=[list(range(NCORES))],
                            ins=[ag_inA.opt()],
                            outs=[ag_outA.opt()],
                        )
                    elif ib == 2:
                        nc.sync.dma_start(ag_inB[:], agPfull[:, 2 * FB :])
                        nc.gpsimd.collective_compute(
                            "AllReduce",
                            ADD,
                            replica_groups=[list(range(NCORES))],
                            ins=[ag_inB.opt()],
                            outs=[ag_outB.opt()],
                        )

                # read-backs AFTER all stagings/triggers so the sync queue
                # never blocks a staging behind an AR wait
                nc.sync.dma_start(agARfull[:, 0 : 2 * FB], ag_outA[:])
                nc.sync.dma_start(agARfull[:, 2 * FB :], ag_outB[:])

                # ====== per-block: b-update, exp, c-broadcast, next-s ======
                nit = it + 1
                last = nit == 2
                s_ps = psA.tile([BL, O, P], f32, tag="sps")
                cexp = cexpp.tile([128, IC, OP], fp16, tag="cexp")
                eT16 = ring1.tile([O, I], fp16, tag=f"eT{it}")
                bT = ring1.tile([O, I], f32, tag=f"bT{it}")
                zps = []
                first_mm = True
                for ib in range(NBLK):
                    blk = slice(ib * FB, (ib + 1) * FB)
                    if bT_prev is None:
                        nc.vector.tensor_scalar_mul(
                            bT[:, blk], agARfull[:, blk], 1.0 / B
                        )
                    else:
                        nc.vector.scalar_tensor_tensor(
                            bT[:, blk],
                            agARfull[:, blk],
                            1.0 / B,
                            bT_prev[:, blk],
                            op0=MUL,
                            op1=ADD,
                        )
                    zp = ring1.tile([O, 1], f32, tag=f"zp{it}{ib}")
                    nc.scalar.activation(
                        eT16[:, blk],
                        bT[:, blk],
                        AF.Exp,
                        bias=zero_b[:O],
                        accum_out=zp[:],
                    )
                    zps.append(zp)
                    if ib == NBLK - 1:
                        # prefetch sqrt for the upcoming squash
                        tsd = ring2.tile([1, 2], f32, tag=f"tsd{nit}")
                        nc.scalar.activation(
                            tsd[:], tscr[:], AF.Sqrt, bias=eps_b[:1]
                        )

                    # unnormalized c broadcast across p: ce[m,(o,p)] = e[o, i]
                    ce = psC.tile([128, QB * OP], f32, tag="ce")
                    for q in range(QB):
                        icx = ib * QB + q
                        nc.tensor.matmul(
                            ce[:, q * OP : (q + 1) * OP],
                            eT16[:, icx * 128 : (icx + 1) * 128],
                            b2h[:],
                            start=True,
                            stop=True,
                        )
                    nc.scalar.copy(
                        out=cexp[:, ib * QB : (ib + 1) * QB, :],
                        in_=ce[:, 0 : QB * OP].rearrange("p (q n) -> p q n", n=OP),
                    )

                    # next-iteration s-matmuls for this i-block
                    for s in range(S):
                        ks = slice(s * IC + ib * QB, s * IC + (ib + 1) * QB)
                        rhs = rhsp.tile([128, QB, OP], fp16, tag="rhs16")
                        nc.vector.tensor_tensor(
                            rhs[:], wsh[:, ks, :], cexp[:, ib * QB : (ib + 1) * QB, :], MUL
                        )
                        for q in range(QB):
                            k = s * IC + ib * QB + q
                            nc.tensor.matmul(
                                s_ps[:],
                                x2h[:, k, :],
                                rhs[:, q, :],
                                start=first_mm,
                                stop=(ib == NBLK - 1 and s == S - 1 and q == QB - 1),
                            )
                            first_mm = False
                bT_prev = bT

                # ====== Z combine + 1/Z broadcast to [BL, O] via the PE ======
                zs1 = ring1.tile([O, 1], f32, tag=f"zs1{it}")
                nc.vector.tensor_tensor(zs1[:], zps[0][:], zps[1][:], ADD)
                zsum = ring1.tile([O, 1], f32, tag=f"zsum{it}")
                nc.vector.tensor_tensor(zsum[:], zs1[:], zps[2][:], ADD)
                zrec = ring1.tile([O, 1], f32, tag=f"zrec{it}")
                nc.vector.reciprocal(zrec[:], zsum[:])
                diag = ring1.tile([O, O], fp16, tag=f"diag{it}")
                nc.vector.tensor_scalar_mul(diag[:], id10[:], zrec[:])
                zb_ps = psC.tile([BL, O], f32, tag="ce")
                nc.tensor.matmul(zb_ps[:], ones64[:], diag[:], start=True, stop=True)
                zb1 = ring1.tile([BL, O], f32, tag=f"zb1{it}")
                nc.vector.tensor_copy(zb1[:], zb_ps[:])
                zb2 = ring1.tile([BL, O], f32, tag=f"zb2{it}")
                nc.vector.tensor_tensor(zb2[:], zb1[:], zb1[:], MUL)

                # ====== squash of the unnormalized s ======
                s_sb = ring1.tile([BL, O, P], f32, tag=f"s_sb{nit}")
                nc.vector.tensor_copy(s_sb[:], s_ps[:])
                s2 = ring1.tile([BL, O, P], f32, tag=f"s2{nit}")
                nc.vector.tensor_tensor(s2[:], s_sb[:], s_sb[:], MUL)
                squn = ring1.tile([BL, O], f32, tag=f"squn{nit}")
                nc.vector.tensor_reduce(
                    squn[:], s2[:], axis=mybir.AxisListType.X, op=ADD
                )
                sq = ring1.tile([BL, O], f32, tag=f"sq{nit}")
                nc.vector.tensor_tensor(sq[:], squn[:], zb2[:], MUL)
                sqs = ring1.tile([BL, O], f32, tag=f"sqs{nit}")
                nc.scalar.activation(sqs[:], sq[:], AF.Sqrt, bias=eps_b[:BL])
                den = ring1.tile([BL, O], f32, tag=f"den{nit}")
                nc.vector.scalar_tensor_tensor(
                    den[:], sq[:], 1.0, sqs[:], op0=ADD, op1=MUL
                )
                rec = ring1.tile([BL, O], f32, tag=f"rec{nit}")
                nc.vector.reciprocal(rec[:], den[:])
                tfac = ring1.tile([BL, O], f32, tag=f"tfac{nit}")
                nc.vector.tensor_tensor(tfac[:], sq[:], rec[:], MUL)
                tf2 = ring1.tile([BL, O], f32, tag=f"tf2{nit}")
                nc.vector.tensor_tensor(tf2[:], tfac[:], zb1[:], MUL)

                if last:
                    v_sb = ring1.tile([BL, O, P], f32, tag="v_sb")
                    nc.vector.tensor_tensor(
                        v_sb[:],
                        s_sb[:],
                        tf2[:, :, None].to_broadcast([BL, O, P]),
                        MUL,
                    )
                    nc.sync.dma_start(vout_d[:], v_sb[:])
                else:
                    nc.vector.tensor_tensor(
                        v8[:, 0:OP].rearrange("b (o p) -> b o p", o=O),
                        s_sb[:],
                        tf2[:, :, None].to_broadcast([BL, O, P]),
                        MUL,
                    )

    nc.compile()
    return nc


def _get_module():
    if "nc" not in _CACHE:
        _CACHE["nc"] = _build_module()
    return _CACHE["nc"]


def _prep_inputs(x, W):
    """Host-side relayouts (free: not counted in HW exec time)."""
    f8 = ml_dtypes.float8_e4m3
    f8w = ml_dtypes.float8_e3m4
    f16 = np.float16
    x = np.ascontiguousarray(np.asarray(x, np.float32))
    W = np.ascontiguousarray(np.asarray(W, np.float32))

    x2 = x.transpose(2, 1, 0).reshape(IS, B)           # [(s,i), b]
    x2t = x2.reshape(KT, 128, B).transpose(1, 0, 2)    # [p, k, b]
    x3 = x.transpose(0, 2, 1).reshape(B, IS)           # [b, (s,i)]
    ws = W.transpose(3, 0, 1, 2).reshape(IS, OP)       # [(s,i), (o,p)]
    wst = ws.reshape(KT, 128, OP).transpose(1, 0, 2)   # [p, k, (o,p)]
    wt = W.transpose(1, 2, 3, 0).reshape(OP, IS)       # [(o,p), (s,i)]

    wt2 = np.zeros((128, 2, IS), np.float32)
    wt2[:, 0, :] = wt[0:128]
    wt2[0:32, 1, :] = wt[128:160]

    sel8 = np.zeros((128, 2, 32), np.float32)
    for o in range(O):
        sel8[o * P : (o + 1) * P, 0, o] = 1.0
    for r in range(32):
        sel8[r, 1, (128 + r) // P] = 1.0

    b2 = np.zeros((O, OP), np.float32)
    for o in range(O):
        b2[o, o * P : (o + 1) * P] = 1.0

    shared = {
        "wsh": np.ascontiguousarray(wst).astype(f16),
        "wt2": wt2.astype(f8w),
        "sel8": sel8.astype(f8),
        "b2h": b2.astype(f16),
        "ones64": np.ones((O, BL), f16),
        "id10": np.eye(O, dtype=f16),
    }
    in_maps = []
    for c in range(NCORES):
        bs = slice(c * BL, (c + 1) * BL)
        m = dict(shared)
        m["x2h"] = np.ascontiguousarray(x2t[:, :, bs]).astype(f16)
        m["x3f"] = np.ascontiguousarray(x3[bs]).astype(f8)
        in_maps.append(m)
    return in_maps


def run(x, W, trace=False, tmpdir=None):
    import jax
    from concourse import bass_utils

    try:
        jax.config.update("jax_compilation_cache_dir", "/tmp/jax_neff_cache")
        jax.config.update("jax_persistent_cache_min_compile_time_secs", 1.0)
    except Exception:
        pass

    nc = _get_module()
    in_maps = _prep_inputs(x, W)
    res = bass_utils.run_bass_kernel_spmd(
        nc, in_maps, core_ids=list(range(NCORES)), trace=trace, tmpdir=tmpdir
    )
    v = np.concatenate([res.results[c]["vout"] for c in range(NCORES)], axis=0)
    return v.reshape(B, O, P).astype(np.float32), res


def kernel(x, W):
    v, _ = run(x, W)
    return v


# revision 6
# speedup vs baseline: 1.1030x; 1.0476x over previous
"""DigitCaps dynamic-routing kernel for Trainium2 (8 NeuronCores, Bass/Tile).

Strategy (pure batch data-parallelism, 64 batch rows per core):
  u_hat (B,1152,10,16) is NEVER materialized. Per routing iteration:

    s[b,(o,p)]   = sum_k x2[k,b] * (e ⊙ Ws)[k,(o,p)]      (72 fp16 K-tile matmuls)
    M2[(o,p),k]  = sum_b v[b,(o,p)] * x3[b,k]             (fp8 matmuls; 160 (o,p)
                   rows live as 128 + 32-padded-to-128 in two PSUM banks)
    agree[o,i]   = Sel^T @ (W ⊙ M2)  with (p,s)-reduction on the PE
                   (one tensor_tensor product + one fp8 DoubleRow Sel matmul
                   per 384-col chunk, accumulated over s in PSUM)

  PIPELINED ROUTING (this version): the agree phase is split into 3 i-blocks
  of 384.  Each block's agree partial is AllReduced independently (3 small
  [10,384] collectives per iteration instead of one [10,1152]), and the next
  iteration's b-update / exp / c-broadcast / s-matmuls for that i-block start
  as soon as its AllReduce lands — overlapping the remaining blocks' agree
  compute and collectives.  This removes the ~25-35us of all-engine idle per
  iteration that the monolithic AllReduce caused.

  UNNORMALIZED SOFTMAX: to decouple blocks, c_ij = exp(b)/Z is NOT normalized
  before the s-matmul; the rhs uses exp(b) directly (b in [-0.03, 0.21], so
  exp(b)~1 is safe in fp16) and the global 1/Z_o is applied at the end:
  a diag(1/Z) [10,10] matmul against a ones[10,64] stationary broadcasts
  1/Z_o to a [64,10] tile, which folds into the squash scalars exactly
  (v = s_un*squn*zb2 / ((1+squn*zb2)*sqrt(squn*zb2+eps)) * zb1).  This is
  mathematically identical to the reference softmax (the shift is 0).

Precision: s-matmuls in fp16 for all 3 iterations; the agree path (v, x3,
products) in fp8e4m3, W-product operand in fp8e3m4 — this error only perturbs
the routing logits b_ij and stays well inside the 2e-2 budget.

Scalar-engine activation tables: exp and sqrt live in different table sets, so
each switch costs ~1.3us. Dummy activations prefetch the upcoming set while
the scalar engine is idle, keeping table loads off the serial chains.
"""
import sys

sys.path.insert(0, "/opt/trn_rl_repo")

import numpy as np
import ml_dtypes

# ---- problem constants (hardcoded per harness contract) ----
B, I, S, O, P = 512, 1152, 8, 10, 16
IS = I * S            # 9216  contraction size, k = s*I + i
OP = O * P            # 160
NCORES = 8
BL = B // NCORES      # 64 batch rows per core
KT = IS // 128        # 72 K-tiles
IC = I // 128         # 9 i-chunks per s-group
FB = 384              # free-chunk width of the agree pipeline (I = 3*FB)
NBLK = 3              # i-blocks per routing iteration (pipelined AllReduce)
QB = IC // NBLK       # 3 k-chunks of 128 per (s, block)
F32MAX = 512          # PSUM bank width in f32 elements

SEL_LAG = 3           # chunks between product issue and its Sel matmul

_CACHE = {}


def _build_module():
    import concourse.bass as bass
    import concourse.mybir as mybir
    import concourse.tile as tile
    from concourse import bacc

    f32 = mybir.dt.float32
    bf16 = mybir.dt.bfloat16
    fp16 = mybir.dt.float16
    fp8 = mybir.dt.float8e4
    fp8w = mybir.dt.float8e3
    MUL = mybir.AluOpType.mult
    ADD = mybir.AluOpType.add
    DR = mybir.MatmulPerfMode.DoubleRow
    AF = mybir.ActivationFunctionType

    nc = bacc.Bacc(
        "TRN2",
        target_bir_lowering=False,
        debug=False,
        num_devices=NCORES,
    )

    # ---- I/O ----
    x2h_d = nc.dram_tensor("x2h", [128, KT, BL], fp16, kind="ExternalInput")
    wsh_d = nc.dram_tensor("wsh", [128, KT, OP], fp16, kind="ExternalInput")
    x3f_d = nc.dram_tensor("x3f", [BL, IS], fp8, kind="ExternalInput")
    wt2_d = nc.dram_tensor("wt2", [128, 2, IS], fp8w, kind="ExternalInput")
    sel8_d = nc.dram_tensor("sel8", [128, 2, 32], fp8, kind="ExternalInput")
    b2h_d = nc.dram_tensor("b2h", [O, OP], fp16, kind="ExternalInput")
    ones64_d = nc.dram_tensor("ones64", [O, BL], fp16, kind="ExternalInput")
    id10_d = nc.dram_tensor("id10", [O, O], fp16, kind="ExternalInput")
    vout_d = nc.dram_tensor("vout", [BL, OP], f32, kind="ExternalOutput")

    with tile.TileContext(nc) as tc:
        with (
            tc.tile_pool(name="const", bufs=1) as const,
            tc.tile_pool(name="rhsbig", bufs=3) as rhsp,
            tc.tile_pool(name="prod", bufs=8) as prodp,
            tc.tile_pool(name="m2c", bufs=2) as m2cp,
            tc.tile_pool(name="cexp", bufs=2) as cexpp,
            tc.tile_pool(name="ring1", bufs=1) as ring1,
            tc.tile_pool(name="ring2", bufs=2) as ring2,
            tc.tile_pool(name="psA", bufs=1, space="PSUM") as psA,
            tc.tile_pool(name="psM", bufs=2, space="PSUM") as psM,
            tc.tile_pool(name="psG", bufs=2, space="PSUM") as psG,
            tc.tile_pool(name="psC", bufs=1, space="PSUM") as psC,
            tc.tile_pool(name="dram", bufs=1, space="DRAM") as dram,
        ):
            # ---------- persistent tiles ----------
            x2h = const.tile([128, KT, BL], fp16)
            wsh = const.tile([128, KT, OP], fp16)
            x3f = const.tile([BL, IS], fp8)
            wt2 = const.tile([128, 2, IS], fp8w)
            sel8 = const.tile([128, 2, 32], fp8)
            b2h = const.tile([O, OP], fp16)
            ones64 = const.tile([O, BL], fp16)
            id10 = const.tile([O, O], fp16)
            v8 = const.tile([BL, 2 * 128], fp8)  # squash out + zero pad

            # Sized warmup AllReduce staged FIRST: it absorbs both the
            # first-collective setup (~11us) AND the inter-core launch skew
            # (~25us) while the input DMAs and iter0 compute run, so the
            # first real agree AllReduce runs at full speed.
            warm_sb = const.tile([O, FB], f32)
            nc.vector.memset(warm_sb[:], 0.0)
            warm_in = dram.tile([O, FB], f32, tag="warm_in")
            warm_out = dram.tile([O, FB], f32, tag="warm_out")
            nc.sync.dma_start(warm_in[:], warm_sb[:])
            nc.gpsimd.collective_compute(
                "AllReduce",
                ADD,
                replica_groups=[list(range(NCORES))],
                ins=[warm_in.opt()],
                outs=[warm_out.opt()],
            )

            # ---------- load inputs (3 parallel queues) ----------
            # scalar/gpsimd: wsh alternating s-groups (the iter0 critical
            # stream); sync: x2h then agree-phase operands.
            nc.gpsimd.dma_start(sel8[:], sel8_d[:])
            nc.gpsimd.dma_start(b2h[:], b2h_d[:])
            nc.gpsimd.dma_start(ones64[:], ones64_d[:])
            nc.gpsimd.dma_start(id10[:], id10_d[:])
            for q0 in range(0, 3):
                ks = slice(q0 * 3, (q0 + 1) * 3)
                nc.scalar.dma_start(wsh[:, ks, :], wsh_d[:, ks, :])
            for s in range(1, S):
                ks = slice(s * IC, (s + 1) * IC)
                eng = nc.scalar if s % 2 == 0 else nc.gpsimd
                eng.dma_start(wsh[:, ks, :], wsh_d[:, ks, :])
            for s in range(S):
                ks = slice(s * IC, (s + 1) * IC)
                nc.sync.dma_start(x2h[:, ks, :], x2h_d[:, ks, :])
            nc.sync.dma_start(x3f[:], x3f_d[:])
            JCH = 3 * FB
            for c0 in range(0, IS, JCH):
                cs = slice(c0, c0 + JCH)
                nc.sync.dma_start(wt2[:, :, cs], wt2_d[:, :, cs])

            # zero pad region of v8 (persists across iterations)
            nc.vector.memset(v8[:, OP:], 0.0)

            # bias APs for activation (float biases need pre-registered consts)
            zero_b = const.tile([128, 1], f32)
            eps_b = const.tile([128, 1], f32)
            nc.vector.memset(zero_b[:], 0.0)
            nc.vector.memset(eps_b[:], 1e-8)
            # scratch for activation-table prefetch dummies
            tscr = const.tile([1, 2], f32)
            nc.vector.memset(tscr[:], 1.0)

            # ---------- iter0 s-matmul phase (uniform c = 1/I) ----------
            s_ps = psA.tile([BL, O, P], f32, tag="sps")
            for s in range(S):
                if s == 0:
                    # prefetch sqrt table while the PE streams
                    tsd = ring2.tile([1, 2], f32, tag="tsd0")
                    nc.scalar.activation(tsd[:], tscr[:], AF.Sqrt, bias=eps_b[:1])
                for icx in range(IC):
                    k = s * IC + icx
                    nc.tensor.matmul(
                        s_ps[:],
                        x2h[:, k, :],
                        wsh[:, k, :],
                        start=(k == 0),
                        stop=(k == KT - 1),
                    )

            # ---------- iter0 squash (exact: fold 1/I) ----------
            s_sb = ring1.tile([BL, O, P], f32, tag="s_sb0")
            nc.vector.tensor_scalar_mul(s_sb[:], s_ps[:], 1.0 / I)
            s2 = ring1.tile([BL, O, P], f32, tag="s20")
            nc.vector.tensor_tensor(s2[:], s_sb[:], s_sb[:], MUL)
            sq = ring1.tile([BL, O], f32, tag="sq0")
            nc.vector.tensor_reduce(sq[:], s2[:], axis=mybir.AxisListType.X, op=ADD)
            sqs = ring1.tile([BL, O], f32, tag="sqs0")
            nc.scalar.activation(sqs[:], sq[:], AF.Sqrt, bias=eps_b[:BL])
            den = ring1.tile([BL, O], f32, tag="den0")
            nc.vector.scalar_tensor_tensor(
                den[:], sq[:], 1.0, sqs[:], op0=ADD, op1=MUL
            )
            rec = ring1.tile([BL, O], f32, tag="rec0")
            nc.vector.reciprocal(rec[:], den[:])
            tfac = ring1.tile([BL, O], f32, tag="tfac0")
            nc.vector.tensor_tensor(tfac[:], sq[:], rec[:], MUL)
            nc.vector.tensor_tensor(
                v8[:, 0:OP].rearrange("b (o p) -> b o p", o=O),
                s_sb[:],
                tfac[:, :, None].to_broadcast([BL, O, P]),
                MUL,
            )

            bT_prev = None  # SBUF (10, I) f32 routing logits

            for it in range(2):  # routing updates
                # =========== agree phase, block-major, pipelined AR ==========
                # Two AllReduces per iteration: A covers i-blocks {0,1}, B
                # covers block {2}.  Collective triggers BLOCK the gpsimd
                # queue until the previous collective completes, so block 2's
                # products must stay off gpsimd (they'd stall behind trig A).
                agPfull = ring1.tile([O, I], f32, tag=f"agP{it}")
                agARfull = ring1.tile([O, I], f32, tag=f"agAR{it}")
                ag_inA = dram.tile([O, 2 * FB], f32, tag=f"aginA{it}")
                ag_outA = dram.tile([O, 2 * FB], f32, tag=f"agoutA{it}")
                ag_inB = dram.tile([O, FB], f32, tag=f"aginB{it}")
                ag_outB = dram.tile([O, FB], f32, tag=f"agoutB{it}")
                for ib in range(NBLK):
                    ag = psG.tile([32, F32MAX], f32, tag="ag")
                    pend = []

                    def emit_sel(pe):
                        pa, s_idx = pe
                        nc.tensor.matmul(
                            ag[:, 0:FB],
                            sel8[:],
                            pa[:],
                            start=(s_idx == 0),
                            stop=(s_idx == S - 1),
                            perf_mode=DR,
                        )

                    for s in range(S):
                        j = s * NBLK + ib
                        fs = slice(j * FB, (j + 1) * FB)
                        m2 = psM.tile([128, 2, F32MAX], f32, tag="m2")
                        nc.tensor.matmul(
                            m2[:, 0, 0:FB],
                            v8[:, 0:128],
                            x3f[:, fs],
                            start=True,
                            stop=True,
                        )
                        nc.tensor.matmul(
                            m2[:, 1, 0:FB],
                            v8[:, 128:256],
                            x3f[:, fs],
                            start=True,
                            stop=True,
                        )
                        if ib == 0 and s == 0:
                            # prefetch exp table while the PE streams the agree
                            # phase (squash's sqrt is done)
                            ted = ring2.tile([1, 2], f32, tag=f"ted{it}")
                            nc.scalar.activation(
                                ted[:], tscr[:], AF.Exp, bias=zero_b[:1]
                            )
                        if len(pend) >= SEL_LAG:
                            emit_sel(pend.pop(0))
                        pa = prodp.tile([128, 2, FB], fp8, tag="prod")
                        if ib < 2 and s in (1, 4):
                            m2c = m2cp.tile([128, 2, FB], bf16, tag="m2c")
                            nc.scalar.copy(out=m2c[:], in_=m2[:, :, 0:FB])
                            nc.gpsimd.tensor_tensor(pa[:], m2c[:], wt2[:, :, fs], MUL)
                        else:
                            nc.vector.tensor_tensor(
                                pa[:], m2[:, :, 0:FB], wt2[:, :, fs], MUL
                            )
                        pend.append((pa, s))
                    while pend:
                        emit_sel(pend.pop(0))

                    blk = slice(ib * FB, (ib + 1) * FB)
                    nc.scalar.copy(out=agPfull[:, blk], in_=ag[0:O, 0:FB])
                    if ib == 1:
                        nc.sync.dma_start(ag_inA[:], agPfull[:, 0 : 2 * FB])
                        nc.gpsimd.collective_compute(
                            "AllReduce",
                            ADD,
                            replica_groups=[list(range(NCORES))],
                            ins=[ag_inA.opt()],
                            outs=[ag_outA.opt()],
                        )
                    elif ib == 2:
                        nc.sync.dma_start(ag_inB[:], agPfull[:, 2 * FB :])
                        nc.gpsimd.collective_compute(
                            "AllReduce",
                            ADD,
                            replica_groups=[list(range(NCORES))],
                            ins=[ag_inB.opt()],
                            outs=[ag_outB.opt()],
                        )

                # read-backs AFTER all stagings/triggers so the sync queue
                # never blocks a staging behind an AR wait
                nc.sync.dma_start(agARfull[:, 0 : 2 * FB], ag_outA[:])
                nc.sync.dma_start(agARfull[:, 2 * FB :], ag_outB[:])

                # ====== per-block: b-update, exp, c-broadcast, next-s ======
                nit = it + 1
                last = nit == 2
                s_ps = psA.tile([BL, O, P], f32, tag="sps")
                cexp = cexpp.tile([128, IC, OP], fp16, tag="cexp")
                eT16 = ring1.tile([O, I], fp16, tag=f"eT{it}")
                bT = ring1.tile([O, I], f32, tag=f"bT{it}")
                zps = []
                first_mm = True
                for ib in range(NBLK):
                    blk = slice(ib * FB, (ib + 1) * FB)
                    if bT_prev is None:
                        nc.vector.tensor_scalar_mul(
                            bT[:, blk], agARfull[:, blk], 1.0 / B
                        )
                    else:
                        nc.vector.scalar_tensor_tensor(
                            bT[:, blk],
                            agARfull[:, blk],
                            1.0 / B,
                            bT_prev[:, blk],
                            op0=MUL,
                            op1=ADD,
                        )
                    zp = ring1.tile([O, 1], f32, tag=f"zp{it}{ib}")
                    nc.scalar.activation(
                        eT16[:, blk],
                        bT[:, blk],
                        AF.Exp,
                        bias=zero_b[:O],
                        accum_out=zp[:],
                    )
                    zps.append(zp)
                    if ib == NBLK - 1:
                        # prefetch sqrt for the upcoming squash
                        tsd = ring2.tile([1, 2], f32, tag=f"tsd{nit}")
                        nc.scalar.activation(
                            tsd[:], tscr[:], AF.Sqrt, bias=eps_b[:1]
                        )

                    # unnormalized c broadcast across p: ce[m,(o,p)] = e[o, i]
                    ce = psC.tile([128, QB * OP], f32, tag="ce")
                    for q in range(QB):
                        icx = ib * QB + q
                        nc.tensor.matmul(
                            ce[:, q * OP : (q + 1) * OP],
                            eT16[:, icx * 128 : (icx + 1) * 128],
                            b2h[:],
                            start=True,
                            stop=True,
                        )
                    nc.scalar.copy(
                        out=cexp[:, ib * QB : (ib + 1) * QB, :],
                        in_=ce[:, 0 : QB * OP].rearrange("p (q n) -> p q n", n=OP),
                    )

                    # next-iteration s-matmuls for this i-block
                    for s in range(S):
                        ks = slice(s * IC + ib * QB, s * IC + (ib + 1) * QB)
                        rhs = rhsp.tile([128, QB, OP], fp16, tag="rhs16")
                        nc.vector.tensor_tensor(
                            rhs[:], wsh[:, ks, :], cexp[:, ib * QB : (ib + 1) * QB, :], MUL
                        )
                        for q in range(QB):
                            k = s * IC + ib * QB + q
                            nc.tensor.matmul(
                                s_ps[:],
                                x2h[:, k, :],
                                rhs[:, q, :],
                                start=first_mm,
                                stop=(ib == NBLK - 1 and s == S - 1 and q == QB - 1),
                            )
                            first_mm = False
                bT_prev = bT

                # ====== Z combine + 1/Z broadcast to [BL, O] via the PE ======
                zs1 = ring1.tile([O, 1], f32, tag=f"zs1{it}")
                nc.vector.tensor_tensor(zs1[:], zps[0][:], zps[1][:], ADD)
                zsum = ring1.tile([O, 1], f32, tag=f"zsum{it}")
                nc.vector.tensor_tensor(zsum[:], zs1[:], zps[2][:], ADD)
                zrec = ring1.tile([O, 1], f32, tag=f"zrec{it}")
                nc.vector.reciprocal(zrec[:], zsum[:])
                diag = ring1.tile([O, O], fp16, tag=f"diag{it}")
                nc.vector.tensor_scalar_mul(diag[:], id10[:], zrec[:])
                zb_ps = psC.tile([BL, O], f32, tag="ce")
                nc.tensor.matmul(zb_ps[:], ones64[:], diag[:], start=True, stop=True)
                zb1 = ring1.tile([BL, O], f32, tag=f"zb1{it}")
                nc.vector.tensor_copy(zb1[:], zb_ps[:])
                zb2 = ring1.tile([BL, O], f32, tag=f"zb2{it}")
                nc.vector.tensor_tensor(zb2[:], zb1[:], zb1[:], MUL)

                # ====== squash of the unnormalized s ======
                s_sb = ring1.tile([BL, O, P], f32, tag=f"s_sb{nit}")
                nc.vector.tensor_copy(s_sb[:], s_ps[:])
                s2 = ring1.tile([BL, O, P], f32, tag=f"s2{nit}")
                nc.vector.tensor_tensor(s2[:], s_sb[:], s_sb[:], MUL)
                squn = ring1.tile([BL, O], f32, tag=f"squn{nit}")
                nc.vector.tensor_reduce(
                    squn[:], s2[:], axis=mybir.AxisListType.X, op=ADD
                )
                sq = ring1.tile([BL, O], f32, tag=f"sq{nit}")
                nc.vector.tensor_tensor(sq[:], squn[:], zb2[:], MUL)
                sqs = ring1.tile([BL, O], f32, tag=f"sqs{nit}")
                nc.scalar.activation(sqs[:], sq[:], AF.Sqrt, bias=eps_b[:BL])
                den = ring1.tile([BL, O], f32, tag=f"den{nit}")
                nc.vector.scalar_tensor_tensor(
                    den[:], sq[:], 1.0, sqs[:], op0=ADD, op1=MUL
                )
                rec = ring1.tile([BL, O], f32, tag=f"rec{nit}")
                nc.vector.reciprocal(rec[:], den[:])
                tfac = ring1.tile([BL, O], f32, tag=f"tfac{nit}")
                nc.vector.tensor_tensor(tfac[:], sq[:], rec[:], MUL)
                tf2 = ring1.tile([BL, O], f32, tag=f"tf2{nit}")
                nc.vector.tensor_tensor(tf2[:], tfac[:], zb1[:], MUL)

                if last:
                    v_sb = ring1.tile([BL, O, P], f32, tag="v_sb")
                    nc.vector.tensor_tensor(
                        v_sb[:],
                        s_sb[:],
                        tf2[:, :, None].to_broadcast([BL, O, P]),
                        MUL,
                    )
                    nc.sync.dma_start(vout_d[:], v_sb[:])
                else:
                    nc.vector.tensor_tensor(
                        v8[:, 0:OP].rearrange("b (o p) -> b o p", o=O),
                        s_sb[:],
                        tf2[:, :, None].to_broadcast([BL, O, P]),
                        MUL,
                    )

    nc.compile()
    return nc


def _get_module():
    if "nc" not in _CACHE:
        _CACHE["nc"] = _build_module()
    return _CACHE["nc"]


def _prep_inputs(x, W):
    """Host-side relayouts (free: not counted in HW exec time)."""
    f8 = ml_dtypes.float8_e4m3
    f8w = ml_dtypes.float8_e3m4
    f16 = np.float16
    x = np.ascontiguousarray(np.asarray(x, np.float32))
    W = np.ascontiguousarray(np.asarray(W, np.float32))

    x2 = x.transpose(2, 1, 0).reshape(IS, B)           # [(s,i), b]
    x2t = x2.reshape(KT, 128, B).transpose(1, 0, 2)    # [p, k, b]
    x3 = x.transpose(0, 2, 1).reshape(B, IS)           # [b, (s,i)]
    ws = W.transpose(3, 0, 1, 2).reshape(IS, OP)       # [(s,i), (o,p)]
    wst = ws.reshape(KT, 128, OP).transpose(1, 0, 2)   # [p, k, (o,p)]
    wt = W.transpose(1, 2, 3, 0).reshape(OP, IS)       # [(o,p), (s,i)]

    wt2 = np.zeros((128, 2, IS), np.float32)
    wt2[:, 0, :] = wt[0:128]
    wt2[0:32, 1, :] = wt[128:160]

    sel8 = np.zeros((128, 2, 32), np.float32)
    for o in range(O):
        sel8[o * P : (o + 1) * P, 0, o] = 1.0
    for r in range(32):
        sel8[r, 1, (128 + r) // P] = 1.0

    b2 = np.zeros((O, OP), np.float32)
    for o in range(O):
        b2[o, o * P : (o + 1) * P] = 1.0

    shared = {
        "wsh": np.ascontiguousarray(wst).astype(f16),
        "wt2": wt2.astype(f8w),
        "sel8": sel8.astype(f8),
        "b2h": b2.astype(f16),
        "ones64": np.ones((O, BL), f16),
        "id10": np.eye(O, dtype=f16),
    }
    in_maps = []
    for c in range(NCORES):
        bs = slice(c * BL, (c + 1) * BL)
        m = dict(shared)
        m["x2h"] = np.ascontiguousarray(x2t[:, :, bs]).astype(f16)
        m["x3f"] = np.ascontiguousarray(x3[bs]).astype(f8)
        in_maps.append(m)
    return in_maps


def run(x, W, trace=False, tmpdir=None):
    import jax
    from concourse import bass_utils

    try:
        jax.config.update("jax_compilation_cache_dir", "/tmp/jax_neff_cache")
        jax.config.update("jax_persistent_cache_min_compile_time_secs", 1.0)
    except Exception:
        pass

    nc = _get_module()
    in_maps = _prep_inputs(x, W)
    res = bass_utils.run_bass_kernel_spmd(
        nc, in_maps, core_ids=list(range(NCORES)), trace=trace, tmpdir=tmpdir
    )
    v = np.concatenate([res.results[c]["vout"] for c in range(NCORES)], axis=0)
    return v.reshape(B, O, P).astype(np.float32), res


def kernel(x, W):
    v, _ = run(x, W)
    return v
